# revision 12
# baseline (speedup 1.0000x reference)
"""Multi-head causal attention (B=2, S=2048, D=1024, H=16) on 8 TRN2 cores.

Sharding: tensor-parallel over heads (4 groups of 4 heads) x data-parallel
over batch (2), one (batch, head-group) pair per core.

Per core:
  - Q/K projections computed transposed (QT/KT: [c, tok], c = head-major
    projection column) so scores can run with head-dim as the contraction.
  - V projection computed in [tok, c] layout, augmented with a ones column
    per head so the attention-weight row sums (softmax denominators) fall
    out of the AV matmul for free.
  - scoresT[k, q] = KT_h.T-slice @ QT_h-slice (K=64 contraction), exp on
    ACT (no max subtraction: scores are O(1) by construction), causal
    masking via precomputed 0/1 tiles multiplied on the diagonal blocks.
  - AV^T accumulated over k-blocks in PSUM; normalized by the broadcast
    reciprocal of the ones-row; W_O row-parallel partial output written as
    outT [dout, tok].
Host side: inputs are pre-transposed/pre-cast, partial outputs summed over
the 4 head-group cores per batch, V-bias and output bias folded into an
effective bias added at gather time (softmax rows sum to 1).
"""

import sys

if "/opt/trn_rl_repo" not in sys.path:
    sys.path.insert(0, "/opt/trn_rl_repo")

import numpy as np
import ml_dtypes

import concourse.bass as bass
import concourse.bacc as bacc
import concourse.tile as tile
from concourse import mybir
from concourse.bass_utils import run_bass_kernel_spmd

F32 = mybir.dt.float32
F32R = mybir.dt.float32r
BF16 = mybir.dt.bfloat16

P = 128
S = 2048          # sequence length
D = 1024          # model dim
C = 256           # projection columns per core (4 heads x 64)
HG = 4            # heads per core
DK = 64           # head dim
ND = 8            # d-blocks of 128 in D
NTOK = 16         # token blocks of 128
NQ = 4            # q tiles of 512
QW = 512


def build_attention_nc(causal: bool):
    nc = bacc.Bacc(None, target_bir_lowering=False)

    xq = nc.dram_tensor("xq", [D, S], BF16, kind="ExternalInput")
    xk = nc.dram_tensor("xk", [D, S], BF16, kind="ExternalInput")
    xv = nc.dram_tensor("xv", [D, S], BF16, kind="ExternalInput")
    wq = nc.dram_tensor("wq", [D, C], BF16, kind="ExternalInput")
    wk = nc.dram_tensor("wk", [D, C], BF16, kind="ExternalInput")
    wv = nc.dram_tensor("wv", [D, C], BF16, kind="ExternalInput")
    wo = nc.dram_tensor("wo", [C, D], BF16, kind="ExternalInput")
    bq = nc.dram_tensor("bq", [P, 2], F32, kind="ExternalInput")
    bk = nc.dram_tensor("bk", [P, 2], F32, kind="ExternalInput")
    masks = nc.dram_tensor("masks", [4, P, QW], BF16, kind="ExternalInput")
    outT = nc.dram_tensor("outT", [D, S], BF16, kind="ExternalOutput")

    with tile.TileContext(nc) as tc:
        from contextlib import ExitStack

        with ExitStack() as ctx:
            const = ctx.enter_context(tc.tile_pool(name="const", bufs=1))
            xp = ctx.enter_context(tc.tile_pool(name="xp", bufs=12))
            resid = ctx.enter_context(tc.tile_pool(name="resid", bufs=1))
            epool = ctx.enter_context(tc.tile_pool(name="epool", bufs=10))
            dpool = ctx.enter_context(tc.tile_pool(name="dpool", bufs=3))
            opool = ctx.enter_context(tc.tile_pool(name="opool", bufs=3))
            ps_mm = ctx.enter_context(tc.tile_pool(name="ps_mm", bufs=3, space="PSUM"))
            ps_s = ctx.enter_context(tc.tile_pool(name="ps_s", bufs=2, space="PSUM"))
            ps_v = ctx.enter_context(tc.tile_pool(name="ps_v", bufs=1, space="PSUM"))

            # ---- constants ----
            wq_t = const.tile([P, ND, C], BF16, name="wq_t")
            wk_t = const.tile([P, ND, C], BF16, name="wk_t")
            wv_t = const.tile([P, ND, C], BF16, name="wv_t")
            for d in range(ND):
                nc.sync.dma_start(out=wq_t[:, d, :], in_=wq[d * P:(d + 1) * P, :])
            for d in range(ND):
                nc.sync.dma_start(out=wk_t[:, d, :], in_=wk[d * P:(d + 1) * P, :])
            for d in range(ND):
                nc.sync.dma_start(out=wv_t[:, d, :], in_=wv[d * P:(d + 1) * P, :])
            wo_r = const.tile([P, 2, D], BF16, name="wo_r")
            nc.sync.dma_start(out=wo_r, in_=wo.rearrange("(n p) d -> p n d", p=P))
            bq_t = const.tile([P, 2], F32, name="bq_t")
            bk_t = const.tile([P, 2], F32, name="bk_t")
            nc.sync.dma_start(out=bq_t, in_=bq[:, :])
            nc.sync.dma_start(out=bk_t, in_=bk[:, :])
            if causal:
                mask_t = const.tile([P, 4, QW], BF16, name="mask_t")
                nc.sync.dma_start(out=mask_t, in_=masks.rearrange("m p f -> p m f"))

            # ---- residents ----
            QT = resid.tile([P, 2, S], BF16, name="QT")
            KT = resid.tile([P, 2, S], BF16, name="KT")
            Vp = resid.tile([P, NTOK, HG, DK + 1], BF16, name="Vp")
            AVT = resid.tile([P, 2, S], BF16, name="AVT")
            ones_t = const.tile([P, 1], BF16, name="ones_t")
            nc.vector.memset(ones_t, 1.0)
            nc.vector.tensor_copy(
                out=Vp[:, :, :, DK:DK + 1],
                in_=ones_t.to_broadcast((P, NTOK, HG, 1)),
            )

            def load_x_tiles(xdram, th):
                xts = []
                for d in range(ND):
                    xt = xp.tile([P, S // 2], BF16, name="x_t")
                    nc.sync.dma_start(
                        out=xt,
                        in_=xdram[d * P:(d + 1) * P, th * (S // 2):(th + 1) * (S // 2)],
                    )
                    xts.append(xt)
                return xts

            def proj_qk(which, th):
                xdram, w_t, b_t, scale = (
                    (xq, wq_t, bq_t, 0.125) if which == "q" else (xk, wk_t, bk_t, 1.0)
                )
                dst = QT if which == "q" else KT
                xts = load_x_tiles(xdram, th)
                for cs in range(2):
                    for t2 in range(2):
                        ps = ps_mm.tile([P, QW], F32, name="mm_ps")
                        for d in range(ND):
                            nc.tensor.matmul(
                                ps,
                                w_t[:, d, cs * P:(cs + 1) * P],
                                xts[d][:, t2 * QW:(t2 + 1) * QW],
                                start=(d == 0),
                                stop=(d == ND - 1),
                            )
                        # (psum * scale) + bias, on DVE (ACT is budgeted for exps)
                        nc.vector.tensor_scalar(
                            dst[:, cs, (th * 2 + t2) * QW:(th * 2 + t2 + 1) * QW],
                            ps,
                            scale,
                            b_t[:, cs:cs + 1],
                            op0=mybir.AluOpType.mult,
                            op1=mybir.AluOpType.add,
                        )

            def proj_v(th):
                xts = load_x_tiles(xv, th)
                for t8 in range(8):
                    ps = ps_mm.tile([P, QW], F32, name="mm_ps")
                    for d in range(ND):
                        nc.tensor.matmul(
                            ps[:, 0:C],
                            xts[d][:, t8 * P:(t8 + 1) * P],
                            wv_t[:, d, :],
                            start=(d == 0),
                            stop=(d == ND - 1),
                        )
                    tok = th * 8 + t8
                    nc.vector.tensor_copy(
                        out=Vp[:, tok, :, 0:DK],
                        in_=ps[:, 0:C].rearrange("p (h e) -> p h e", h=HG),
                    )

            def attn_group(j, h):
                nkb = 4 * j + 4 if causal else NTOK
                hp, hr = divmod(h, 2)
                rows = slice(hr * DK, hr * DK + DK)
                # scores + exp (+mask) for all k-blocks first, paired to
                # amortize ACT overhead; AV matmuls after, so the PE never
                # waits per-block on the ACT->DVE chain.
                ets = []
                for pp in range(nkb // 2):
                    sps = ps_s.tile([P, 2 * QW], F32, name="s_ps")
                    for half in (0, 1):
                        kb = 2 * pp + half
                        nc.tensor.matmul(
                            sps[:, half * QW:(half + 1) * QW],
                            KT[rows, hp, kb * P:(kb + 1) * P],
                            QT[rows, hp, j * QW:(j + 1) * QW],
                            start=True,
                            stop=True,
                        )
                    et = epool.tile([P, 2 * QW], BF16, name="e_t")
                    nc.scalar.activation(et, sps, mybir.ActivationFunctionType.Exp)
                    if causal and 2 * pp >= 4 * j:
                        d0 = 2 * pp - 4 * j
                        nc.vector.tensor_mul(
                            et.rearrange("p (m f) -> p m f", m=2),
                            et.rearrange("p (m f) -> p m f", m=2),
                            mask_t[:, d0:d0 + 2, :],
                        )
                    ets.append(et)
                avp = ps_v.tile([P, QW], F32, name="av_ps")
                for kb in range(nkb):
                    nc.tensor.matmul(
                        avp[0:DK + 1, :],
                        Vp[:, kb, h, :],
                        ets[kb // 2][:, (kb % 2) * QW:(kb % 2 + 1) * QW],
                        start=(kb == 0),
                        stop=(kb == nkb - 1),
                    )
                avs = dpool.tile([DK, QW], F32, name="avs_t")
                nc.vector.tensor_copy(out=avs, in_=avp[0:DK, :])
                den = dpool.tile([1, QW], F32, name="den_t")
                nc.vector.tensor_copy(out=den, in_=avp[DK:DK + 1, :])
                rec = dpool.tile([1, QW], F32, name="rec_t")
                nc.vector.reciprocal_approx_fast(out=rec, in_=den)
                bc = dpool.tile([DK, QW], F32, name="bc_t")
                nc.gpsimd.partition_broadcast(bc, rec)
                nc.vector.tensor_mul(
                    AVT[hr * DK:(hr + 1) * DK, hp, j * QW:(j + 1) * QW],
                    avs,
                    bc,
                )

            def final_proj(qn):
                for m in range(ND):
                    ps = ps_mm.tile([P, QW], F32, name="mm_ps")
                    for cs in range(2):
                        nc.tensor.matmul(
                            ps,
                            wo_r[:, cs, m * P:(m + 1) * P],
                            AVT[:, cs, qn * QW:(qn + 1) * QW],
                            start=(cs == 0),
                            stop=(cs == 1),
                        )
                    ot = opool.tile([P, QW], BF16, name="o_t")
                    nc.vector.tensor_copy(out=ot, in_=ps)
                    nc.sync.dma_start(
                        out=outT[m * P:(m + 1) * P, qn * QW:(qn + 1) * QW],
                        in_=ot,
                    )

            # Emission order interleaves projection halves with attention so
            # the big ACT exp load overlaps PE projection matmuls.
            proj_qk("q", 0)
            proj_qk("k", 0)
            proj_v(0)
            for j in (0, 1):
                for h in range(HG):
                    attn_group(j, h)
            proj_qk("q", 1)
            proj_qk("k", 1)
            proj_v(1)
            for j in (2, 3):
                for h in range(HG):
                    attn_group(j, h)
            for qn in range(NQ):
                final_proj(qn)

    nc.compile()
    return nc


_NC_CACHE = {}


def _get_nc(causal: bool):
    if causal not in _NC_CACHE:
        _NC_CACHE[causal] = build_attention_nc(causal)
    return _NC_CACHE[causal]


def _causal_mask_tiles():
    # masks[delta][kk, qq] = 1.0 where (k0 + kk) <= (q0 + qq), k0 - q0 = 128*delta
    m = np.zeros((4, P, QW), np.float32)
    kk = np.arange(P)[:, None]
    qq = np.arange(QW)[None, :]
    for d in range(4):
        m[d] = (qq >= kk + d * P).astype(np.float32)
    return m


def kernel(query, key, value, mask, Wq, bq, Wk, bk, Wv, bv, Wo, bo):
    query = np.asarray(query, np.float32)
    key = np.asarray(key, np.float32)
    value = np.asarray(value, np.float32)
    Wq = np.asarray(Wq, np.float32)
    Wk = np.asarray(Wk, np.float32)
    Wv = np.asarray(Wv, np.float32)
    Wo = np.asarray(Wo, np.float32)
    bq = np.asarray(bq, np.float32)
    bk = np.asarray(bk, np.float32)
    bv = np.asarray(bv, np.float32)
    bo = np.asarray(bo, np.float32)
    mask_np = np.asarray(mask)

    causal = bool(mask_np.any())
    nc = _get_nc(causal)

    mask_tiles = (_causal_mask_tiles() if causal else np.ones((4, P, QW), np.float32)).astype(ml_dtypes.bfloat16)

    # Per-batch transposed inputs in bf16 (shared by the 4 cores of a batch).
    xqT = [np.ascontiguousarray(query[b].T).astype(ml_dtypes.bfloat16) for b in range(2)]
    xkT = [np.ascontiguousarray(key[b].T).astype(ml_dtypes.bfloat16) for b in range(2)]
    xvT = [np.ascontiguousarray(value[b].T).astype(ml_dtypes.bfloat16) for b in range(2)]

    # torch Linear: y = x @ W.T; W.T is (in, out) = (d, c).
    WqT = np.ascontiguousarray(Wq.T)
    WkT = np.ascontiguousarray(Wk.T)
    WvT = np.ascontiguousarray(Wv.T)
    WoT = np.ascontiguousarray(Wo.T)

    in_maps = []
    for core in range(8):
        b, g = divmod(core, 4)
        cols = slice(g * C, (g + 1) * C)
        in_maps.append({
            "xq": xqT[b],
            "xk": xkT[b],
            "xv": xvT[b],
            "wq": np.ascontiguousarray(WqT[:, cols]).astype(ml_dtypes.bfloat16),
            "wk": np.ascontiguousarray(WkT[:, cols]).astype(ml_dtypes.bfloat16),
            "wv": np.ascontiguousarray(WvT[:, cols]).astype(ml_dtypes.bfloat16),
            "wo": np.ascontiguousarray(WoT[cols, :]).astype(ml_dtypes.bfloat16),
            "bq": np.ascontiguousarray((bq[cols] / 8.0).reshape(2, P).T),
            "bk": np.ascontiguousarray(bk[cols].reshape(2, P).T),
            "masks": mask_tiles,
        })

    res = run_bass_kernel_spmd(nc, in_maps, core_ids=list(range(8)))

    # softmax rows sum to 1, so the V bias contributes bv @ Wo.T to every row.
    bo_eff = bo + bv @ Wo.T
    out = np.empty((2, S, D), np.float32)
    for b in range(2):
        acc = res.results[b * 4]["outT"].astype(np.float32)
        for g in range(1, 4):
            acc += res.results[b * 4 + g]["outT"].astype(np.float32)
        out[b] = acc.T.astype(np.float32) + bo_eff
    return out


# revision 13
# speedup vs baseline: 1.0093x; 1.0093x over previous
"""Multi-head causal attention (B=2, S=2048, D=1024, H=16) on 8 TRN2 cores.

Sharding: tensor-parallel over heads (4 groups of 4 heads) x data-parallel
over batch (2), one (batch, head-group) pair per core.

Per core:
  - Q/K projections computed transposed (QT/KT: [c, tok], c = head-major
    projection column) so scores can run with head-dim as the contraction.
  - V projection computed in [tok, c] layout, augmented with a ones column
    per head so the attention-weight row sums (softmax denominators) fall
    out of the AV matmul for free.
  - scoresT[k, q] = KT_h.T-slice @ QT_h-slice (K=64 contraction), exp on
    ACT (no max subtraction: scores are O(1) by construction), causal
    masking via precomputed 0/1 tiles multiplied on the diagonal blocks.
  - AV^T accumulated over k-blocks in PSUM; normalized by the broadcast
    reciprocal of the ones-row; W_O row-parallel partial output written as
    outT [dout, tok].
Host side: inputs are pre-transposed/pre-cast, partial outputs summed over
the 4 head-group cores per batch, V-bias and output bias folded into an
effective bias added at gather time (softmax rows sum to 1).
"""

import sys

if "/opt/trn_rl_repo" not in sys.path:
    sys.path.insert(0, "/opt/trn_rl_repo")

import numpy as np
import ml_dtypes

import concourse.bass as bass
import concourse.bacc as bacc
import concourse.tile as tile
from concourse import mybir
from concourse.bass_utils import run_bass_kernel_spmd

F32 = mybir.dt.float32
F32R = mybir.dt.float32r
BF16 = mybir.dt.bfloat16

P = 128
S = 2048          # sequence length
D = 1024          # model dim
C = 256           # projection columns per core (4 heads x 64)
HG = 4            # heads per core
DK = 64           # head dim
ND = 8            # d-blocks of 128 in D
NTOK = 16         # token blocks of 128
NQ = 4            # q tiles of 512
QW = 512


def build_attention_nc(causal: bool):
    nc = bacc.Bacc(None, target_bir_lowering=False)

    xq = nc.dram_tensor("xq", [D, S], BF16, kind="ExternalInput")
    xk = nc.dram_tensor("xk", [D, S], BF16, kind="ExternalInput")
    xv = nc.dram_tensor("xv", [D, S], BF16, kind="ExternalInput")
    wq = nc.dram_tensor("wq", [D, C], BF16, kind="ExternalInput")
    wk = nc.dram_tensor("wk", [D, C], BF16, kind="ExternalInput")
    wv = nc.dram_tensor("wv", [D, C], BF16, kind="ExternalInput")
    wo = nc.dram_tensor("wo", [C, D], BF16, kind="ExternalInput")
    bq = nc.dram_tensor("bq", [P, 2], F32, kind="ExternalInput")
    bk = nc.dram_tensor("bk", [P, 2], F32, kind="ExternalInput")
    masks = nc.dram_tensor("masks", [4, P, QW], BF16, kind="ExternalInput")
    outT = nc.dram_tensor("outT", [D, S], BF16, kind="ExternalOutput")

    with tile.TileContext(nc) as tc:
        from contextlib import ExitStack

        with ExitStack() as ctx:
            const = ctx.enter_context(tc.tile_pool(name="const", bufs=1))
            xp = ctx.enter_context(tc.tile_pool(name="xp", bufs=12))
            resid = ctx.enter_context(tc.tile_pool(name="resid", bufs=1))
            epool = ctx.enter_context(tc.tile_pool(name="epool", bufs=10))
            dpool = ctx.enter_context(tc.tile_pool(name="dpool", bufs=4))
            opool = ctx.enter_context(tc.tile_pool(name="opool", bufs=8))
            ps_mm = ctx.enter_context(tc.tile_pool(name="ps_mm", bufs=3, space="PSUM"))
            ps_s = ctx.enter_context(tc.tile_pool(name="ps_s", bufs=2, space="PSUM"))
            ps_v = ctx.enter_context(tc.tile_pool(name="ps_v", bufs=1, space="PSUM"))

            # ---- constants ----
            wq_t = const.tile([P, ND, C], BF16, name="wq_t")
            wk_t = const.tile([P, ND, C], BF16, name="wk_t")
            wv_t = const.tile([P, ND, C], BF16, name="wv_t")
            for d in range(ND):
                nc.sync.dma_start(out=wq_t[:, d, :], in_=wq[d * P:(d + 1) * P, :])
            for d in range(ND):
                nc.sync.dma_start(out=wk_t[:, d, :], in_=wk[d * P:(d + 1) * P, :])
            for d in range(ND):
                nc.sync.dma_start(out=wv_t[:, d, :], in_=wv[d * P:(d + 1) * P, :])
            wo_r = const.tile([P, 2, D], BF16, name="wo_r")
            nc.sync.dma_start(out=wo_r, in_=wo.rearrange("(n p) d -> p n d", p=P))
            bq_t = const.tile([P, 2], F32, name="bq_t")
            bk_t = const.tile([P, 2], F32, name="bk_t")
            nc.sync.dma_start(out=bq_t, in_=bq[:, :])
            nc.sync.dma_start(out=bk_t, in_=bk[:, :])
            if causal:
                mask_t = const.tile([P, 4, QW], BF16, name="mask_t")
                nc.sync.dma_start(out=mask_t, in_=masks.rearrange("m p f -> p m f"))

            # ---- residents ----
            QT = resid.tile([P, 2, S], BF16, name="QT")
            KT = resid.tile([P, 2, S], BF16, name="KT")
            Vp = resid.tile([P, NTOK, HG, DK + 1], BF16, name="Vp")
            AVT = resid.tile([P, 2, S], BF16, name="AVT")
            ones_t = const.tile([P, 1], BF16, name="ones_t")
            nc.vector.memset(ones_t, 1.0)
            nc.vector.tensor_copy(
                out=Vp[:, :, :, DK:DK + 1],
                in_=ones_t.to_broadcast((P, NTOK, HG, 1)),
            )

            def load_x_tiles(xdram, th):
                xts = []
                for d in range(ND):
                    xt = xp.tile([P, S // 2], BF16, name="x_t")
                    nc.sync.dma_start(
                        out=xt,
                        in_=xdram[d * P:(d + 1) * P, th * (S // 2):(th + 1) * (S // 2)],
                    )
                    xts.append(xt)
                return xts

            def proj_qk(which, th):
                xdram, w_t, b_t, scale = (
                    (xq, wq_t, bq_t, 0.125) if which == "q" else (xk, wk_t, bk_t, 1.0)
                )
                dst = QT if which == "q" else KT
                xts = load_x_tiles(xdram, th)
                for cs in range(2):
                    for t2 in range(2):
                        ps = ps_mm.tile([P, QW], F32, name="mm_ps")
                        for d in range(ND):
                            nc.tensor.matmul(
                                ps,
                                w_t[:, d, cs * P:(cs + 1) * P],
                                xts[d][:, t2 * QW:(t2 + 1) * QW],
                                start=(d == 0),
                                stop=(d == ND - 1),
                            )
                        # (psum * scale) + bias, on DVE (ACT is budgeted for exps)
                        nc.vector.tensor_scalar(
                            dst[:, cs, (th * 2 + t2) * QW:(th * 2 + t2 + 1) * QW],
                            ps,
                            scale,
                            b_t[:, cs:cs + 1],
                            op0=mybir.AluOpType.mult,
                            op1=mybir.AluOpType.add,
                        )

            def proj_v(th):
                xts = load_x_tiles(xv, th)
                for t8 in range(8):
                    ps = ps_mm.tile([P, QW], F32, name="mm_ps")
                    for d in range(ND):
                        nc.tensor.matmul(
                            ps[:, 0:C],
                            xts[d][:, t8 * P:(t8 + 1) * P],
                            wv_t[:, d, :],
                            start=(d == 0),
                            stop=(d == ND - 1),
                        )
                    tok = th * 8 + t8
                    nc.vector.tensor_copy(
                        out=Vp[:, tok, :, 0:DK],
                        in_=ps[:, 0:C].rearrange("p (h e) -> p h e", h=HG),
                    )

            def attn_group(j, h):
                nkb = 4 * j + 4 if causal else NTOK
                hp, hr = divmod(h, 2)
                rows = slice(hr * DK, hr * DK + DK)
                # scores + exp (+mask) for all k-blocks first, paired to
                # amortize ACT overhead; AV matmuls after, so the PE never
                # waits per-block on the ACT->DVE chain.
                ets = []
                for pp in range(nkb // 2):
                    sps = ps_s.tile([P, 2 * QW], F32, name="s_ps")
                    for half in (0, 1):
                        kb = 2 * pp + half
                        nc.tensor.matmul(
                            sps[:, half * QW:(half + 1) * QW],
                            KT[rows, hp, kb * P:(kb + 1) * P],
                            QT[rows, hp, j * QW:(j + 1) * QW],
                            start=True,
                            stop=True,
                        )
                    et = epool.tile([P, 2 * QW], BF16, name="e_t")
                    nc.scalar.activation(et, sps, mybir.ActivationFunctionType.Exp)
                    if causal and 2 * pp >= 4 * j:
                        d0 = 2 * pp - 4 * j
                        nc.vector.tensor_mul(
                            et.rearrange("p (m f) -> p m f", m=2),
                            et.rearrange("p (m f) -> p m f", m=2),
                            mask_t[:, d0:d0 + 2, :],
                        )
                    ets.append(et)
                avp = ps_v.tile([P, QW], F32, name="av_ps")
                for kb in range(nkb):
                    nc.tensor.matmul(
                        avp[0:DK + 1, :],
                        Vp[:, kb, h, :],
                        ets[kb // 2][:, (kb % 2) * QW:(kb % 2 + 1) * QW],
                        start=(kb == 0),
                        stop=(kb == nkb - 1),
                    )
                avs = dpool.tile([DK, QW], F32, name="avs_t")
                nc.vector.tensor_copy(out=avs, in_=avp[0:DK, :])
                den = dpool.tile([1, QW], F32, name="den_t")
                nc.vector.tensor_copy(out=den, in_=avp[DK:DK + 1, :])
                rec = dpool.tile([1, QW], F32, name="rec_t")
                nc.vector.reciprocal_approx_fast(out=rec, in_=den)
                bc = dpool.tile([DK, QW], F32, name="bc_t")
                nc.gpsimd.partition_broadcast(bc, rec)
                nc.vector.tensor_mul(
                    AVT[hr * DK:(hr + 1) * DK, hp, j * QW:(j + 1) * QW],
                    avs,
                    bc,
                )

            def final_proj(qn):
                for m in range(ND):
                    ps = ps_mm.tile([P, QW], F32, name="mm_ps")
                    for cs in range(2):
                        nc.tensor.matmul(
                            ps,
                            wo_r[:, cs, m * P:(m + 1) * P],
                            AVT[:, cs, qn * QW:(qn + 1) * QW],
                            start=(cs == 0),
                            stop=(cs == 1),
                        )
                    ot = opool.tile([P, QW], BF16, name="o_t")
                    nc.vector.tensor_copy(out=ot, in_=ps)
                    nc.sync.dma_start(
                        out=outT[m * P:(m + 1) * P, qn * QW:(qn + 1) * QW],
                        in_=ot,
                    )

            # Emission order interleaves projection halves with attention so
            # the big ACT exp load overlaps PE projection matmuls.
            proj_qk("q", 0)
            proj_qk("k", 0)
            proj_v(0)
            for j in (0, 1):
                for h in range(HG):
                    attn_group(j, h)
            proj_qk("q", 1)
            proj_qk("k", 1)
            proj_v(1)
            for j in (2, 3):
                for h in range(HG):
                    attn_group(j, h)
            for qn in range(NQ):
                final_proj(qn)

    nc.compile()
    return nc


_NC_CACHE = {}


def _get_nc(causal: bool):
    if causal not in _NC_CACHE:
        _NC_CACHE[causal] = build_attention_nc(causal)
    return _NC_CACHE[causal]


def _causal_mask_tiles():
    # masks[delta][kk, qq] = 1.0 where (k0 + kk) <= (q0 + qq), k0 - q0 = 128*delta
    m = np.zeros((4, P, QW), np.float32)
    kk = np.arange(P)[:, None]
    qq = np.arange(QW)[None, :]
    for d in range(4):
        m[d] = (qq >= kk + d * P).astype(np.float32)
    return m


def kernel(query, key, value, mask, Wq, bq, Wk, bk, Wv, bv, Wo, bo):
    query = np.asarray(query, np.float32)
    key = np.asarray(key, np.float32)
    value = np.asarray(value, np.float32)
    Wq = np.asarray(Wq, np.float32)
    Wk = np.asarray(Wk, np.float32)
    Wv = np.asarray(Wv, np.float32)
    Wo = np.asarray(Wo, np.float32)
    bq = np.asarray(bq, np.float32)
    bk = np.asarray(bk, np.float32)
    bv = np.asarray(bv, np.float32)
    bo = np.asarray(bo, np.float32)
    mask_np = np.asarray(mask)

    causal = bool(mask_np.any())
    nc = _get_nc(causal)

    mask_tiles = (_causal_mask_tiles() if causal else np.ones((4, P, QW), np.float32)).astype(ml_dtypes.bfloat16)

    # Per-batch transposed inputs in bf16 (shared by the 4 cores of a batch).
    xqT = [np.ascontiguousarray(query[b].T).astype(ml_dtypes.bfloat16) for b in range(2)]
    xkT = [np.ascontiguousarray(key[b].T).astype(ml_dtypes.bfloat16) for b in range(2)]
    xvT = [np.ascontiguousarray(value[b].T).astype(ml_dtypes.bfloat16) for b in range(2)]

    # torch Linear: y = x @ W.T; W.T is (in, out) = (d, c).
    WqT = np.ascontiguousarray(Wq.T)
    WkT = np.ascontiguousarray(Wk.T)
    WvT = np.ascontiguousarray(Wv.T)
    WoT = np.ascontiguousarray(Wo.T)

    in_maps = []
    for core in range(8):
        b, g = divmod(core, 4)
        cols = slice(g * C, (g + 1) * C)
        in_maps.append({
            "xq": xqT[b],
            "xk": xkT[b],
            "xv": xvT[b],
            "wq": np.ascontiguousarray(WqT[:, cols]).astype(ml_dtypes.bfloat16),
            "wk": np.ascontiguousarray(WkT[:, cols]).astype(ml_dtypes.bfloat16),
            "wv": np.ascontiguousarray(WvT[:, cols]).astype(ml_dtypes.bfloat16),
            "wo": np.ascontiguousarray(WoT[cols, :]).astype(ml_dtypes.bfloat16),
            "bq": np.ascontiguousarray((bq[cols] / 8.0).reshape(2, P).T),
            "bk": np.ascontiguousarray(bk[cols].reshape(2, P).T),
            "masks": mask_tiles,
        })

    res = run_bass_kernel_spmd(nc, in_maps, core_ids=list(range(8)))

    # softmax rows sum to 1, so the V bias contributes bv @ Wo.T to every row.
    bo_eff = bo + bv @ Wo.T
    out = np.empty((2, S, D), np.float32)
    for b in range(2):
        acc = res.results[b * 4]["outT"].astype(np.float32)
        for g in range(1, 4):
            acc += res.results[b * 4 + g]["outT"].astype(np.float32)
        out[b] = acc.T.astype(np.float32) + bo_eff
    return out
